# revision 1
# baseline (speedup 1.0000x reference)
"""Trainium2 Bass kernel for nn_BidirectionalAttention (B=2, N=2048, D=2048, H=16).

Head-parallel tensor sharding across 8 NeuronCores (2 heads/core):
  phase A: qkv projection from x^T (rope applied on natural layout, then
           PE-transpose q,k into [head_dim, seq] layout), intermediates to DRAM
  phase B: per (batch, head): transposed attention scores s^T[k,q] = k^T.T @ q^T,
           exp on ScalarE, softmax denominator via ones-matmul partition sum,
           unnormalized attn @ v accumulated transposed, scaled by 1/sum
  phase C: output projection partial = av^T.T @ wo_rows per core
Host: shard/transpose/pre-round inputs, sum the 8 partial outputs (the
"all-reduce after wo" done at gather time).

Matmuls run in float32r (tf32-like: 11-bit mantissa, full-rate PE) by default;
set KMM_DT=f32 for full-precision fp32 matmuls (4x slower PE).
"""

import os
import sys

sys.path.insert(0, "/opt/trn_rl_repo")

import numpy as np

B, SEQ, DIM, NHEAD, DH = 2, 2048, 2048, 16, 128
HL = NHEAD // 8  # heads per core = 2
NCORES = 8
NT = B * SEQ  # 4096 flattened rows
SCALE = 1.0 / np.sqrt(DH)

_PROG = {}


def _round_f32r(a):
    """Round fp32 array to fp32r (tf32-like): 8-bit exp, 11-bit stored mantissa,
    low 12 bits zero. Round-to-nearest-even."""
    b = np.ascontiguousarray(a, dtype=np.float32).view(np.uint32).astype(np.uint64)
    r = ((b + 0x7FF + ((b >> 12) & 1)) & np.uint64(0xFFFFF000)).astype(np.uint32)
    return r.view(np.float32)


def _build(mm_f32r: bool):
    import concourse.tile as tile
    from concourse import bacc, mybir

    f32 = mybir.dt.float32
    f32r = mybir.dt.float32r
    Exp = mybir.ActivationFunctionType.Exp
    dmm = f32r if mm_f32r else f32

    nc = bacc.Bacc("TRN2", target_bir_lowering=False, debug=False, num_devices=NCORES)

    xt_d = nc.dram_tensor("xt", [DIM, NT], dmm, kind="ExternalInput")
    wqk_d = nc.dram_tensor("wqk", [DIM, 4 * DH], dmm, kind="ExternalInput")
    wv_d = nc.dram_tensor("wv", [DIM, HL * DH], dmm, kind="ExternalInput")
    wo_d = nc.dram_tensor("wo_r", [HL * DH, DIM], dmm, kind="ExternalInput")
    cos_d = nc.dram_tensor("cos2", [NT, 2 * 64], dmm, kind="ExternalInput")
    sin_d = nc.dram_tensor("sin2", [NT, 2 * 64], dmm, kind="ExternalInput")
    ident_d = nc.dram_tensor("ident", [128, 128], dmm, kind="ExternalInput")
    ones_d = nc.dram_tensor("ones", [128, 1], dmm, kind="ExternalInput")
    onesrow_d = nc.dram_tensor("onesrow", [1, 128], dmm, kind="ExternalInput")
    out_d = nc.dram_tensor("out_p", [NT, DIM], f32, kind="ExternalOutput")

    qt_d = nc.dram_tensor("q_t", [HL, DH, NT], dmm)
    kt_d = nc.dram_tensor("k_t", [HL, DH, NT], dmm)
    vn_d = nc.dram_tensor("v_n", [NT, HL * DH], dmm)
    av_d = nc.dram_tensor("av_t", [HL, DH, NT], dmm)

    with tile.TileContext(nc) as tc:
        with (
            nc.allow_low_precision(reason="fp32r (tf32-like) matmul pipeline"),
            tc.tile_pool(name="const", bufs=1) as cp,
        ):
            ident = cp.tile([128, 128], dmm)
            nc.sync.dma_start(ident, ident_d[:, :])
            ones = cp.tile([128, 1], dmm)
            nc.sync.dma_start(ones, ones_d[:, :])
            onesrow = cp.tile([1, 128], dmm)
            nc.sync.dma_start(onesrow, onesrow_d[:, :])
            wo_sb = cp.tile([128, HL, DIM], dmm)
            nc.sync.dma_start(wo_sb, wo_d.rearrange("(j p) o -> p j o", p=128))

            # ---------------- Phase A: qkv projection + rope + transpose ----
            with (
                tc.tile_pool(name="aconst", bufs=1) as ac,
                tc.tile_pool(name="axs", bufs=2) as axs,
                tc.tile_pool(name="awork", bufs=3) as aw,
                tc.tile_pool(name="apsum", bufs=2, space="PSUM") as aps,
                tc.tile_pool(name="atps", bufs=2, space="PSUM") as atp,
            ):
                wqk_sb = ac.tile([128, 16, 4 * DH], dmm)
                nc.sync.dma_start(wqk_sb, wqk_d.rearrange("(c p) m -> p c m", p=128))
                wv_sb = ac.tile([128, 16, HL * DH], dmm)
                nc.sync.dma_start(wv_sb, wv_d.rearrange("(c p) m -> p c m", p=128))
                cos_sb = ac.tile([128, 32, 128], dmm)
                nc.sync.dma_start(cos_sb, cos_d.rearrange("(i p) f -> p i f", p=128))
                sin_sb = ac.tile([128, 32, 128], dmm)
                nc.sync.dma_start(sin_sb, sin_d.rearrange("(i p) f -> p i f", p=128))

                for sl in range(NT // 512):
                    n0 = sl * 512
                    xs = axs.tile([128, 16, 512], dmm, tag="xs")
                    nc.sync.dma_start(
                        xs, xt_d.rearrange("(c p) n -> p c n", p=128)[:, :, n0 : n0 + 512]
                    )
                    for st in range(4):
                        i = sl * 4 + st
                        g0 = n0 + st * 128
                        xsl = xs[:, :, st * 128 : (st + 1) * 128]
                        qkps = aps.tile([128, 4 * DH], f32, tag="qk")
                        for cc in range(16):
                            nc.tensor.matmul(
                                qkps,
                                xsl[:, cc, :],
                                wqk_sb[:, cc, :],
                                start=(cc == 0),
                                stop=(cc == 15),
                            )
                        vps = aps.tile([128, HL * DH], f32, tag="v")
                        for cc in range(16):
                            nc.tensor.matmul(
                                vps,
                                xsl[:, cc, :],
                                wv_sb[:, cc, :],
                                start=(cc == 0),
                                stop=(cc == 15),
                            )
                        vsb = aw.tile([128, HL * DH], dmm, tag="vsb")
                        nc.scalar.copy(vsb, vps)
                        nc.sync.dma_start(vn_d[g0 : g0 + 128, :], vsb)

                        # rope: cols [q0 q1 k0 k1], each 128 = [64 even | 64 odd]
                        rt = aw.tile([128, 4 * DH], dmm, tag="rt")
                        ca = cos_sb[:, i, :].rearrange("p (t f) -> p t f", t=2)
                        sa = sin_sb[:, i, :].rearrange("p (t f) -> p t f", t=2)
                        for g in range(2):
                            blk = qkps[:, g * 256 : (g + 1) * 256].rearrange(
                                "p (t h f) -> p t h f", t=2, h=2
                            )
                            rbl = rt[:, g * 256 : (g + 1) * 256].rearrange(
                                "p (t h f) -> p t h f", t=2, h=2
                            )
                            ev, od = blk[:, :, 0, :], blk[:, :, 1, :]
                            tA = aw.tile([128, 2, 64], f32, tag="tA")
                            tB = aw.tile([128, 2, 64], f32, tag="tB")
                            nc.vector.tensor_mul(tA, od, sa)
                            nc.vector.tensor_mul(tB, ev, ca)
                            nc.vector.tensor_sub(rbl[:, :, 0, :], tB, tA)
                            tC = aw.tile([128, 2, 64], f32, tag="tC")
                            tD = aw.tile([128, 2, 64], f32, tag="tD")
                            nc.vector.tensor_mul(tC, ev, sa)
                            nc.vector.tensor_mul(tD, od, ca)
                            nc.vector.tensor_add(rbl[:, :, 1, :], tD, tC)

                        for t in range(4):
                            tp = atp.tile([128, 128], dmm, tag="tp")
                            nc.tensor.transpose(tp, rt[:, t * 128 : (t + 1) * 128], ident)
                            tsb = aw.tile([128, 128], dmm, tag="tsb")
                            nc.vector.tensor_copy(tsb, tp)
                            dst = (qt_d if t < 2 else kt_d)[t % 2, :, g0 : g0 + 128]
                            nc.sync.dma_start(dst, tsb)

            # ---------------- Phase B: attention per (batch, local head) ----
            with (
                tc.tile_pool(name="bin", bufs=2) as bi,
                tc.tile_pool(name="bprobs", bufs=2) as bp,
                tc.tile_pool(name="bwork", bufs=3) as bw,
                tc.tile_pool(name="bs", bufs=3, space="PSUM") as bs,
                tc.tile_pool(name="bsum", bufs=1, space="PSUM") as bsm,
                tc.tile_pool(name="brb", bufs=1, space="PSUM") as brb,
                tc.tile_pool(name="bav", bufs=2, space="PSUM") as bav,
            ):
                for b in range(B):
                    for j in range(HL):
                        kt_sb = bi.tile([128, SEQ], dmm, tag="kt")
                        nc.sync.dma_start(kt_sb, kt_d[j, :, b * SEQ : (b + 1) * SEQ])
                        qt_sb = bi.tile([128, SEQ], dmm, tag="qt")
                        nc.sync.dma_start(qt_sb, qt_d[j, :, b * SEQ : (b + 1) * SEQ])
                        v_sb = bi.tile([128, 16, DH], dmm, tag="v")
                        nc.sync.dma_start(
                            v_sb,
                            vn_d[
                                b * SEQ : (b + 1) * SEQ, j * DH : (j + 1) * DH
                            ].rearrange("(c p) d -> p c d", p=128),
                        )
                        for qt_i in range(4):
                            q0 = qt_i * 512
                            probs = bp.tile([128, 16, 512], dmm, tag="probs")
                            for kt_i in range(16):
                                sps = bs.tile([128, 512], f32, tag="s")
                                nc.tensor.matmul(
                                    sps,
                                    kt_sb[:, kt_i * 128 : (kt_i + 1) * 128],
                                    qt_sb[:, q0 : q0 + 512],
                                    start=True,
                                    stop=True,
                                )
                                nc.scalar.activation(probs[:, kt_i, :], sps, Exp)
                            sum_ps = bsm.tile([1, 512], f32, tag="sum")
                            for kt_i in range(16):
                                nc.tensor.matmul(
                                    sum_ps,
                                    ones,
                                    probs[:, kt_i, :],
                                    start=(kt_i == 0),
                                    stop=(kt_i == 15),
                                )
                            rv = bw.tile([1, 512], dmm, tag="rv")
                            nc.vector.reciprocal(rv, sum_ps)
                            rbc_ps = brb.tile([128, 512], f32, tag="rbc")
                            nc.tensor.matmul(rbc_ps, onesrow, rv, start=True, stop=True)
                            rbc = bw.tile([128, 512], f32, tag="rbcsb")
                            nc.vector.tensor_copy(rbc, rbc_ps)
                            avps = bav.tile([128, 512], f32, tag="av")
                            for cc in range(16):
                                nc.tensor.matmul(
                                    avps,
                                    v_sb[:, cc, :],
                                    probs[:, cc, :],
                                    start=(cc == 0),
                                    stop=(cc == 15),
                                )
                            av_sb = bw.tile([128, 512], dmm, tag="avsb")
                            nc.vector.tensor_mul(av_sb, avps, rbc)
                            nc.sync.dma_start(
                                av_d[j, :, b * SEQ + q0 : b * SEQ + q0 + 512], av_sb
                            )

            # ---------------- Phase C: output projection -------------------
            with (
                tc.tile_pool(name="cin", bufs=3) as ci,
                tc.tile_pool(name="cwork", bufs=2) as cw,
                tc.tile_pool(name="cps", bufs=4, space="PSUM") as cps,
            ):
                for i in range(NT // 128):
                    g0 = i * 128
                    av_ch = ci.tile([128, HL, 128], dmm, tag="avch")
                    for j in range(HL):
                        nc.sync.dma_start(av_ch[:, j, :], av_d[j, :, g0 : g0 + 128])
                    ot = cw.tile([128, DIM], f32, tag="ot")
                    for do in range(4):
                        ops = cps.tile([128, 512], f32, tag="o")
                        for j in range(HL):
                            nc.tensor.matmul(
                                ops,
                                av_ch[:, j, :],
                                wo_sb[:, j, do * 512 : (do + 1) * 512],
                                start=(j == 0),
                                stop=(j == 1),
                            )
                        nc.vector.tensor_copy(ot[:, do * 512 : (do + 1) * 512], ops)
                    nc.sync.dma_start(out_d[g0 : g0 + 128, :], ot)

    nc.compile()
    return nc


def _get_prog():
    mm_f32r = os.environ.get("KMM_DT", "f32r") == "f32r"
    key = ("prog", mm_f32r)
    if key not in _PROG:
        _PROG[key] = _build(mm_f32r)
    return _PROG[key], mm_f32r


def _shard(x, freqs_cis, wqkv, wo, mm_f32r):
    rnd = _round_f32r if mm_f32r else (lambda a: np.ascontiguousarray(a, np.float32))
    x = np.asarray(x, dtype=np.float32)
    freqs_cis = np.asarray(freqs_cis, dtype=np.float32)
    wqkv = np.asarray(wqkv, dtype=np.float32)
    wo = np.asarray(wo, dtype=np.float32)

    xt = rnd(x.reshape(NT, DIM).T)

    cos = freqs_cis[:, :, 0]  # [SEQ, 64]
    sin = freqs_cis[:, :, 1]
    cosb = np.concatenate([cos] * B, axis=0)  # [NT, 64], row n = b*SEQ + pos
    sinb = np.concatenate([sin] * B, axis=0)
    cos2 = rnd(np.concatenate([cosb, cosb], axis=1))  # [NT, 128] dup halves
    sin2 = rnd(np.concatenate([sinb, sinb], axis=1))

    perm = np.concatenate([np.arange(0, DH, 2), np.arange(1, DH, 2)])  # de-interleave
    consts = {
        "ident": np.eye(128, dtype=np.float32),
        "ones": np.ones((128, 1), np.float32),
        "onesrow": np.ones((1, 128), np.float32),
    }
    in_maps = []
    for c in range(NCORES):
        h0 = c * HL
        wq = [wqkv[:, h * DH : (h + 1) * DH][:, perm] * SCALE for h in (h0, h0 + 1)]
        wk = [wqkv[:, DIM + h * DH : DIM + (h + 1) * DH][:, perm] for h in (h0, h0 + 1)]
        wqk_c = rnd(np.concatenate(wq + wk, axis=1))  # [DIM, 512]
        wv_c = rnd(wqkv[:, 2 * DIM + h0 * DH : 2 * DIM + (h0 + HL) * DH])  # [DIM, 256]
        wo_c = rnd(wo[h0 * DH : (h0 + HL) * DH, :])  # [256, DIM]
        in_maps.append(
            {
                "xt": xt,
                "wqk": wqk_c,
                "wv": wv_c,
                "wo_r": wo_c,
                "cos2": cos2,
                "sin2": sin2,
                **consts,
            }
        )
    return in_maps


def _run(in_maps, trace=False, **kw):
    from concourse.bass_utils import run_bass_kernel_spmd

    prog, _ = _get_prog()
    return run_bass_kernel_spmd(prog, in_maps, list(range(NCORES)), trace=trace, **kw)


def kernel(x, freqs_cis, wqkv, wo):
    _, mm_f32r = _get_prog()
    in_maps = _shard(x, freqs_cis, wqkv, wo, mm_f32r)
    res = _run(in_maps, trace=False)
    acc = np.zeros((NT, DIM), dtype=np.float32)
    for c in range(NCORES):
        acc += res.results[c]["out_p"]
    return acc.reshape(B, SEQ, DIM)


# revision 2
# speedup vs baseline: 1.1038x; 1.1038x over previous
"""Trainium2 Bass kernel for nn_BidirectionalAttention (B=2, N=2048, D=2048, H=16).

Head-parallel tensor sharding across 8 NeuronCores (2 heads/core):
  phase A: qkv projection from x^T (rope applied on natural layout, then
           PE-transpose q,k into [head_dim, seq] layout), intermediates to DRAM
  phase B: per (batch, head): transposed attention scores s^T[k,q] = k^T.T @ q^T,
           exp on ScalarE, softmax denominator via ones-matmul partition sum
           (broadcast back to 128 partitions with a K=1 matmul, then fast
           reciprocal), unnormalized attn @ v accumulated transposed, scaled
  phase C: output projection partial = av^T.T @ wo_rows per core (interleaved
           with phase B per batch; av stays SBUF-resident)
Host: shard/transpose/pre-round inputs, sum the 8 partial outputs (the
"all-reduce after wo" done at gather time).

Matmuls run in float32r (tf32-like: 11-bit mantissa, full-rate PE) by default;
set KMM_DT=f32 for full-precision fp32 matmuls (4x slower PE).
"""

import os
import sys

sys.path.insert(0, "/opt/trn_rl_repo")

import numpy as np

B, SEQ, DIM, NHEAD, DH = 2, 2048, 2048, 16, 128
HL = NHEAD // 8  # heads per core = 2
NCORES = 8
NT = B * SEQ  # 4096 flattened rows
SCALE = 1.0 / np.sqrt(DH)

_PROG = {}


def _round_f32r(a):
    """Round fp32 array to fp32r (tf32-like): 8-bit exp, 11-bit stored mantissa,
    low 12 bits zero. Round-to-nearest-even."""
    b = np.ascontiguousarray(a, dtype=np.float32).view(np.uint32).astype(np.uint64)
    r = ((b + 0x7FF + ((b >> 12) & 1)) & np.uint64(0xFFFFF000)).astype(np.uint32)
    return r.view(np.float32)


def _build(mm_f32r: bool):
    import concourse.tile as tile
    from concourse import bacc, mybir

    f32 = mybir.dt.float32
    f32r = mybir.dt.float32r
    Exp = mybir.ActivationFunctionType.Exp
    dmm = f32r if mm_f32r else f32

    nc = bacc.Bacc("TRN2", target_bir_lowering=False, debug=False, num_devices=NCORES)

    xt_d = nc.dram_tensor("xt", [DIM, NT], dmm, kind="ExternalInput")
    wqk_d = nc.dram_tensor("wqk", [DIM, 4 * DH], dmm, kind="ExternalInput")
    wv_d = nc.dram_tensor("wv", [DIM, HL * DH], dmm, kind="ExternalInput")
    wo_d = nc.dram_tensor("wo_r", [HL * DH, DIM], dmm, kind="ExternalInput")
    cos_d = nc.dram_tensor("cos2", [NT, 2 * 64], dmm, kind="ExternalInput")
    sin_d = nc.dram_tensor("sin2", [NT, 2 * 64], dmm, kind="ExternalInput")
    ident_d = nc.dram_tensor("ident", [128, 128], dmm, kind="ExternalInput")
    ones_d = nc.dram_tensor("ones", [128, 1], dmm, kind="ExternalInput")
    onesrow_d = nc.dram_tensor("onesrow", [1, 128], dmm, kind="ExternalInput")
    out_d = nc.dram_tensor("out_p", [NT, DIM], f32, kind="ExternalOutput")

    qt_d = nc.dram_tensor("q_t", [HL, DH, NT], dmm)
    kt_d = nc.dram_tensor("k_t", [HL, DH, NT], dmm)
    vn_d = nc.dram_tensor("v_n", [NT, HL * DH], dmm)

    with tile.TileContext(nc) as tc:
        with (
            nc.allow_low_precision(reason="fp32r (tf32-like) matmul pipeline"),
            tc.tile_pool(name="const", bufs=1) as cp,
        ):
            ident = cp.tile([128, 128], dmm)
            nc.sync.dma_start(ident, ident_d[:, :])
            ones = cp.tile([128, 1], dmm)
            nc.sync.dma_start(ones, ones_d[:, :])
            onesrow = cp.tile([1, 128], dmm)
            nc.sync.dma_start(onesrow, onesrow_d[:, :])
            wo_sb = cp.tile([128, HL, DIM], dmm)
            nc.sync.dma_start(wo_sb, wo_d.rearrange("(j p) o -> p j o", p=128))

            # ---------------- Phase A: qkv projection + rope + transpose ----
            with (
                tc.tile_pool(name="aconst", bufs=1) as ac,
                tc.tile_pool(name="axs", bufs=2) as axs,
                tc.tile_pool(name="awork", bufs=3) as aw,
                tc.tile_pool(name="apsum", bufs=2, space="PSUM") as aps,
                tc.tile_pool(name="atps", bufs=2, space="PSUM") as atp,
            ):
                wqk_sb = ac.tile([128, 16, 4 * DH], dmm)
                nc.sync.dma_start(wqk_sb, wqk_d.rearrange("(c p) m -> p c m", p=128))
                wv_sb = ac.tile([128, 16, HL * DH], dmm)
                nc.sync.dma_start(wv_sb, wv_d.rearrange("(c p) m -> p c m", p=128))
                cos_sb = ac.tile([128, 32, 128], dmm)
                nc.sync.dma_start(cos_sb, cos_d.rearrange("(i p) f -> p i f", p=128))
                sin_sb = ac.tile([128, 32, 128], dmm)
                nc.sync.dma_start(sin_sb, sin_d.rearrange("(i p) f -> p i f", p=128))

                for sl in range(NT // 512):
                    n0 = sl * 512
                    xs = axs.tile([128, 16, 512], dmm, tag="xs")
                    nc.sync.dma_start(
                        xs, xt_d.rearrange("(c p) n -> p c n", p=128)[:, :, n0 : n0 + 512]
                    )
                    for st in range(4):
                        i = sl * 4 + st
                        g0 = n0 + st * 128
                        xsl = xs[:, :, st * 128 : (st + 1) * 128]
                        qkps = aps.tile([128, 4 * DH], f32, tag="qk")
                        for cc in range(16):
                            nc.tensor.matmul(
                                qkps,
                                xsl[:, cc, :],
                                wqk_sb[:, cc, :],
                                start=(cc == 0),
                                stop=(cc == 15),
                            )
                        vps = aps.tile([128, HL * DH], f32, tag="v")
                        for cc in range(16):
                            nc.tensor.matmul(
                                vps,
                                xsl[:, cc, :],
                                wv_sb[:, cc, :],
                                start=(cc == 0),
                                stop=(cc == 15),
                            )
                        vsb = aw.tile([128, HL * DH], dmm, tag="vsb")
                        nc.scalar.copy(vsb, vps)
                        nc.sync.dma_start(vn_d[g0 : g0 + 128, :], vsb)

                        # rope: cols [q0 q1 k0 k1], each 128 = [64 even | 64 odd]
                        rt = aw.tile([128, 4 * DH], dmm, tag="rt")
                        ca = cos_sb[:, i, :].rearrange("p (t f) -> p t f", t=2)
                        sa = sin_sb[:, i, :].rearrange("p (t f) -> p t f", t=2)
                        for g in range(2):
                            blk = qkps[:, g * 256 : (g + 1) * 256].rearrange(
                                "p (t h f) -> p t h f", t=2, h=2
                            )
                            rbl = rt[:, g * 256 : (g + 1) * 256].rearrange(
                                "p (t h f) -> p t h f", t=2, h=2
                            )
                            ev, od = blk[:, :, 0, :], blk[:, :, 1, :]
                            tA = aw.tile([128, 2, 64], f32, tag="tA")
                            tB = aw.tile([128, 2, 64], f32, tag="tB")
                            nc.vector.tensor_mul(tA, od, sa)
                            nc.vector.tensor_mul(tB, ev, ca)
                            nc.vector.tensor_sub(rbl[:, :, 0, :], tB, tA)
                            tC = aw.tile([128, 2, 64], f32, tag="tC")
                            tD = aw.tile([128, 2, 64], f32, tag="tD")
                            nc.vector.tensor_mul(tC, ev, sa)
                            nc.vector.tensor_mul(tD, od, ca)
                            nc.vector.tensor_add(rbl[:, :, 1, :], tD, tC)

                        for t in range(4):
                            tp = atp.tile([128, 128], dmm, tag="tp")
                            nc.tensor.transpose(tp, rt[:, t * 128 : (t + 1) * 128], ident)
                            tsb = aw.tile([128, 128], dmm, tag="tsb")
                            nc.scalar.copy(tsb, tp)
                            dst = (qt_d if t < 2 else kt_d)[t % 2, :, g0 : g0 + 128]
                            nc.sync.dma_start(dst, tsb)

            # ---------- Phase B+C: attention + output projection ------------
            with (
                tc.tile_pool(name="bin", bufs=2) as bi,
                tc.tile_pool(name="bprobs", bufs=2) as bp,
                tc.tile_pool(name="bwork", bufs=3) as bw,
                tc.tile_pool(name="bavres", bufs=3) as bav_sb,
                tc.tile_pool(name="cot", bufs=2) as cot,
                tc.tile_pool(name="bs", bufs=2, space="PSUM") as bs,
                tc.tile_pool(name="bsum", bufs=1, space="PSUM") as bsm,
                tc.tile_pool(name="brb", bufs=1, space="PSUM") as brb,
                tc.tile_pool(name="bav", bufs=2, space="PSUM") as bav,
            ):
                avres = {}
                for b in range(B):
                    for j in range(HL):
                        kt_sb = bi.tile([128, SEQ], dmm, tag="kt")
                        nc.sync.dma_start(kt_sb, kt_d[j, :, b * SEQ : (b + 1) * SEQ])
                        qt_sb = bi.tile([128, SEQ], dmm, tag="qt")
                        nc.sync.dma_start(qt_sb, qt_d[j, :, b * SEQ : (b + 1) * SEQ])
                        v_sb = bi.tile([128, 16, DH], dmm, tag="v")
                        nc.sync.dma_start(
                            v_sb,
                            vn_d[
                                b * SEQ : (b + 1) * SEQ, j * DH : (j + 1) * DH
                            ].rearrange("(c p) d -> p c d", p=128),
                        )
                        av_r = bav_sb.tile([128, SEQ], dmm, tag="avres")
                        avres[(b, j)] = av_r
                        for qt_i in range(4):
                            q0 = qt_i * 512
                            probs = bp.tile([128, 16, 512], dmm, tag="probs")
                            for kp in range(8):
                                sps = bs.tile([128, 2, 512], f32, tag="s")
                                for u in range(2):
                                    kt_i = 2 * kp + u
                                    nc.tensor.matmul(
                                        sps[:, u, :],
                                        kt_sb[:, kt_i * 128 : (kt_i + 1) * 128],
                                        qt_sb[:, q0 : q0 + 512],
                                        start=True,
                                        stop=True,
                                    )
                                nc.scalar.activation(
                                    probs[:, 2 * kp : 2 * kp + 2, :], sps, Exp
                                )
                            sum_ps = bsm.tile([1, 512], f32, tag="sum")
                            for kt_i in range(16):
                                nc.tensor.matmul(
                                    sum_ps,
                                    ones,
                                    probs[:, kt_i, :],
                                    start=(kt_i == 0),
                                    stop=(kt_i == 15),
                                )
                            sum_sb = bw.tile([1, 512], dmm, tag="sumsb")
                            nc.vector.tensor_copy(sum_sb, sum_ps)
                            rbc_ps = brb.tile([128, 512], f32, tag="rbc")
                            nc.tensor.matmul(rbc_ps, onesrow, sum_sb, start=True, stop=True)
                            rbc = bw.tile([128, 512], f32, tag="rbcsb")
                            nc.vector.reciprocal_approx_fast(rbc, rbc_ps)
                            avps = bav.tile([128, 512], f32, tag="av")
                            for cc in range(16):
                                nc.tensor.matmul(
                                    avps,
                                    v_sb[:, cc, :],
                                    probs[:, cc, :],
                                    start=(cc == 0),
                                    stop=(cc == 15),
                                )
                            nc.vector.tensor_mul(av_r[:, q0 : q0 + 512], avps, rbc)

                    # phase C for this batch: partial out = av^T.T @ wo_rows
                    for nl in range(SEQ // 128):
                        g0 = b * SEQ + nl * 128
                        ot = cot.tile([128, DIM], f32, tag="ot")
                        for do in range(4):
                            ops = bav.tile([128, 512], f32, tag="av")
                            for j in range(HL):
                                nc.tensor.matmul(
                                    ops,
                                    avres[(b, j)][:, nl * 128 : (nl + 1) * 128],
                                    wo_sb[:, j, do * 512 : (do + 1) * 512],
                                    start=(j == 0),
                                    stop=(j == 1),
                                )
                            nc.vector.tensor_copy(ot[:, do * 512 : (do + 1) * 512], ops)
                        nc.sync.dma_start(out_d[g0 : g0 + 128, :], ot)

    nc.compile()
    return nc


def _get_prog():
    mm_f32r = os.environ.get("KMM_DT", "f32r") == "f32r"
    key = ("prog", mm_f32r)
    if key not in _PROG:
        _PROG[key] = _build(mm_f32r)
    return _PROG[key], mm_f32r


def _shard(x, freqs_cis, wqkv, wo, mm_f32r):
    rnd = _round_f32r if mm_f32r else (lambda a: np.ascontiguousarray(a, np.float32))
    x = np.asarray(x, dtype=np.float32)
    freqs_cis = np.asarray(freqs_cis, dtype=np.float32)
    wqkv = np.asarray(wqkv, dtype=np.float32)
    wo = np.asarray(wo, dtype=np.float32)

    xt = rnd(x.reshape(NT, DIM).T)

    cos = freqs_cis[:, :, 0]  # [SEQ, 64]
    sin = freqs_cis[:, :, 1]
    cosb = np.concatenate([cos] * B, axis=0)  # [NT, 64], row n = b*SEQ + pos
    sinb = np.concatenate([sin] * B, axis=0)
    cos2 = rnd(np.concatenate([cosb, cosb], axis=1))  # [NT, 128] dup halves
    sin2 = rnd(np.concatenate([sinb, sinb], axis=1))

    perm = np.concatenate([np.arange(0, DH, 2), np.arange(1, DH, 2)])  # de-interleave
    consts = {
        "ident": np.eye(128, dtype=np.float32),
        "ones": np.ones((128, 1), np.float32),
        "onesrow": np.ones((1, 128), np.float32),
    }
    in_maps = []
    for c in range(NCORES):
        h0 = c * HL
        wq = [wqkv[:, h * DH : (h + 1) * DH][:, perm] * SCALE for h in (h0, h0 + 1)]
        wk = [wqkv[:, DIM + h * DH : DIM + (h + 1) * DH][:, perm] for h in (h0, h0 + 1)]
        wqk_c = rnd(np.concatenate(wq + wk, axis=1))  # [DIM, 512]
        wv_c = rnd(wqkv[:, 2 * DIM + h0 * DH : 2 * DIM + (h0 + HL) * DH])  # [DIM, 256]
        wo_c = rnd(wo[h0 * DH : (h0 + HL) * DH, :])  # [256, DIM]
        in_maps.append(
            {
                "xt": xt,
                "wqk": wqk_c,
                "wv": wv_c,
                "wo_r": wo_c,
                "cos2": cos2,
                "sin2": sin2,
                **consts,
            }
        )
    return in_maps


def _run(in_maps, trace=False, **kw):
    from concourse.bass_utils import run_bass_kernel_spmd

    prog, _ = _get_prog()
    return run_bass_kernel_spmd(prog, in_maps, list(range(NCORES)), trace=trace, **kw)


def kernel(x, freqs_cis, wqkv, wo):
    _, mm_f32r = _get_prog()
    in_maps = _shard(x, freqs_cis, wqkv, wo, mm_f32r)
    res = _run(in_maps, trace=False)
    acc = np.zeros((NT, DIM), dtype=np.float32)
    for c in range(NCORES):
        acc += res.results[c]["out_p"]
    return acc.reshape(B, SEQ, DIM)


# revision 3
# speedup vs baseline: 1.1321x; 1.0256x over previous
"""Trainium2 Bass kernel for nn_BidirectionalAttention (B=2, N=2048, D=2048, H=16).

Head-parallel tensor sharding across 8 NeuronCores (2 heads/core):
  phase A: qkv projection from x^T (rope applied on natural layout, then
           PE-transpose q,k into [head_dim, seq] layout), intermediates to DRAM
  phase B: per (batch, head): transposed attention scores s^T[k,q] = k^T.T @ q^T,
           exp on ScalarE, softmax denominator via ones-matmul partition sum
           (broadcast back to 128 partitions with a K=1 matmul, then fast
           reciprocal), unnormalized attn @ v accumulated transposed, scaled
  phase C: output projection partial = av^T.T @ wo_rows per core (interleaved
           with phase B per batch; av stays SBUF-resident)
Host: shard/transpose/pre-round inputs, sum the 8 partial outputs (the
"all-reduce after wo" done at gather time).

Matmuls run in float32r (tf32-like: 11-bit mantissa, full-rate PE) by default;
set KMM_DT=f32 for full-precision fp32 matmuls (4x slower PE).
"""

import os
import sys

sys.path.insert(0, "/opt/trn_rl_repo")

import numpy as np

B, SEQ, DIM, NHEAD, DH = 2, 2048, 2048, 16, 128
HL = NHEAD // 8  # heads per core = 2
NCORES = 8
NT = B * SEQ  # 4096 flattened rows
SCALE = 1.0 / np.sqrt(DH)

_PROG = {}


def _round_f32r(a):
    """Round fp32 array to fp32r (tf32-like): 8-bit exp, 11-bit stored mantissa,
    low 12 bits zero. Round-to-nearest-even."""
    b = np.ascontiguousarray(a, dtype=np.float32).view(np.uint32).astype(np.uint64)
    r = ((b + 0x7FF + ((b >> 12) & 1)) & np.uint64(0xFFFFF000)).astype(np.uint32)
    return r.view(np.float32)


def _build(mm_f32r: bool):
    import concourse.tile as tile
    from concourse import bacc, mybir

    f32 = mybir.dt.float32
    f32r = mybir.dt.float32r
    Exp = mybir.ActivationFunctionType.Exp
    dmm = f32r if mm_f32r else f32

    nc = bacc.Bacc("TRN2", target_bir_lowering=False, debug=False, num_devices=NCORES)

    xt_d = nc.dram_tensor("xt", [DIM, NT], dmm, kind="ExternalInput")
    wqk_d = nc.dram_tensor("wqk", [DIM, 4 * DH], dmm, kind="ExternalInput")
    wv_d = nc.dram_tensor("wv", [DIM, HL * DH], dmm, kind="ExternalInput")
    wo_d = nc.dram_tensor("wo_r", [HL * DH, DIM], dmm, kind="ExternalInput")
    cos_d = nc.dram_tensor("cos2", [NT, 2 * 64], dmm, kind="ExternalInput")
    sin_d = nc.dram_tensor("sin2", [NT, 2 * 64], dmm, kind="ExternalInput")
    ident_d = nc.dram_tensor("ident", [128, 128], dmm, kind="ExternalInput")
    ones_d = nc.dram_tensor("ones", [128, 1], dmm, kind="ExternalInput")
    onesrow_d = nc.dram_tensor("onesrow", [1, 128], dmm, kind="ExternalInput")
    out_d = nc.dram_tensor("out_p", [NT, DIM], f32, kind="ExternalOutput")

    qt_d = nc.dram_tensor("q_t", [HL, DH, NT], dmm)
    kt_d = nc.dram_tensor("k_t", [HL, DH, NT], dmm)
    vn_d = nc.dram_tensor("v_n", [NT, HL * DH], dmm)

    with tile.TileContext(nc) as tc:
        with (
            nc.allow_low_precision(reason="fp32r (tf32-like) matmul pipeline"),
            tc.tile_pool(name="const", bufs=1) as cp,
        ):
            ident = cp.tile([128, 128], dmm)
            ones = cp.tile([128, 1], dmm)
            onesrow = cp.tile([1, 128], dmm)
            wo_sb = cp.tile([128, HL, DIM], dmm)

            # ---------------- Phase A: qkv projection + rope + transpose ----
            with (
                tc.tile_pool(name="aconst", bufs=1) as ac,
                tc.tile_pool(name="axs", bufs=2) as axs,
                tc.tile_pool(name="awork", bufs=3) as aw,
                tc.tile_pool(name="apsum", bufs=2, space="PSUM") as aps,
                tc.tile_pool(name="atps", bufs=2, space="PSUM") as atp,
            ):
                wqk_sb = ac.tile([128, 16, 4 * DH], dmm)
                wqk_src = wqk_d.rearrange("(c p) m -> p c m", p=128)
                wv_sb = ac.tile([128, 16, HL * DH], dmm)
                wv_src = wv_d.rearrange("(c p) m -> p c m", p=128)
                for cc in range(16):
                    nc.sync.dma_start(wqk_sb[:, cc, :], wqk_src[:, cc, :])
                for cc in range(16):
                    nc.sync.dma_start(wv_sb[:, cc, :], wv_src[:, cc, :])
                cos_sb = ac.tile([128, 32, 128], dmm)
                nc.sync.dma_start(cos_sb, cos_d.rearrange("(i p) f -> p i f", p=128))
                sin_sb = ac.tile([128, 32, 128], dmm)
                nc.sync.dma_start(sin_sb, sin_d.rearrange("(i p) f -> p i f", p=128))
                nc.sync.dma_start(ident, ident_d[:, :])
                nc.sync.dma_start(ones, ones_d[:, :])
                nc.sync.dma_start(onesrow, onesrow_d[:, :])
                nc.sync.dma_start(wo_sb, wo_d.rearrange("(j p) o -> p j o", p=128))

                for sl in range(NT // 512):
                    n0 = sl * 512
                    xs = axs.tile([128, 16, 512], dmm, tag="xs")
                    xt_src = xt_d.rearrange("(c p) n -> p c n", p=128)[:, :, n0 : n0 + 512]
                    for cg in range(4):
                        nc.sync.dma_start(
                            xs[:, 4 * cg : 4 * cg + 4, :], xt_src[:, 4 * cg : 4 * cg + 4, :]
                        )
                    for st in range(4):
                        i = sl * 4 + st
                        g0 = n0 + st * 128
                        xsl = xs[:, :, st * 128 : (st + 1) * 128]
                        qkps = aps.tile([128, 4 * DH], f32, tag="qk")
                        for cc in range(16):
                            nc.tensor.matmul(
                                qkps,
                                xsl[:, cc, :],
                                wqk_sb[:, cc, :],
                                start=(cc == 0),
                                stop=(cc == 15),
                            )
                        vps = aps.tile([128, HL * DH], f32, tag="v")
                        for cc in range(16):
                            nc.tensor.matmul(
                                vps,
                                xsl[:, cc, :],
                                wv_sb[:, cc, :],
                                start=(cc == 0),
                                stop=(cc == 15),
                            )
                        vsb = aw.tile([128, HL * DH], dmm, tag="vsb")
                        nc.scalar.copy(vsb, vps)
                        nc.sync.dma_start(vn_d[g0 : g0 + 128, :], vsb)

                        # rope: cols [q0 q1 k0 k1], each 128 = [64 even | 64 odd]
                        rt = aw.tile([128, 4 * DH], dmm, tag="rt")
                        ca = cos_sb[:, i, :].rearrange("p (t f) -> p t f", t=2)
                        sa = sin_sb[:, i, :].rearrange("p (t f) -> p t f", t=2)
                        for g in range(2):
                            blk = qkps[:, g * 256 : (g + 1) * 256].rearrange(
                                "p (t h f) -> p t h f", t=2, h=2
                            )
                            rbl = rt[:, g * 256 : (g + 1) * 256].rearrange(
                                "p (t h f) -> p t h f", t=2, h=2
                            )
                            ev, od = blk[:, :, 0, :], blk[:, :, 1, :]
                            tA = aw.tile([128, 2, 64], f32, tag="tA")
                            tB = aw.tile([128, 2, 64], f32, tag="tB")
                            nc.vector.tensor_mul(tA, od, sa)
                            nc.vector.tensor_mul(tB, ev, ca)
                            nc.vector.tensor_sub(rbl[:, :, 0, :], tB, tA)
                            tC = aw.tile([128, 2, 64], f32, tag="tC")
                            tD = aw.tile([128, 2, 64], f32, tag="tD")
                            nc.vector.tensor_mul(tC, ev, sa)
                            nc.vector.tensor_mul(tD, od, ca)
                            nc.vector.tensor_add(rbl[:, :, 1, :], tD, tC)

                        for t in range(4):
                            tp = atp.tile([128, 128], dmm, tag="tp")
                            nc.tensor.transpose(tp, rt[:, t * 128 : (t + 1) * 128], ident)
                            tsb = aw.tile([128, 128], dmm, tag="tsb")
                            nc.scalar.copy(tsb, tp)
                            dst = (qt_d if t < 2 else kt_d)[t % 2, :, g0 : g0 + 128]
                            nc.sync.dma_start(dst, tsb)

            # ---------- Phase B+C: attention + output projection ------------
            with (
                tc.tile_pool(name="bin", bufs=2) as bi,
                tc.tile_pool(name="bprobs", bufs=2) as bp,
                tc.tile_pool(name="bwork", bufs=3) as bw,
                tc.tile_pool(name="bavres", bufs=3) as bav_sb,
                tc.tile_pool(name="cot", bufs=2) as cot,
                tc.tile_pool(name="bs", bufs=2, space="PSUM") as bs,
                tc.tile_pool(name="bsum", bufs=1, space="PSUM") as bsm,
                tc.tile_pool(name="brb", bufs=1, space="PSUM") as brb,
                tc.tile_pool(name="bav", bufs=2, space="PSUM") as bav,
            ):
                avres = {}
                for b in range(B):
                    for j in range(HL):
                        kt_sb = bi.tile([128, SEQ], dmm, tag="kt")
                        nc.sync.dma_start(kt_sb, kt_d[j, :, b * SEQ : (b + 1) * SEQ])
                        qt_sb = bi.tile([128, SEQ], dmm, tag="qt")
                        nc.sync.dma_start(qt_sb, qt_d[j, :, b * SEQ : (b + 1) * SEQ])
                        v_sb = bi.tile([128, 16, DH], dmm, tag="v")
                        nc.sync.dma_start(
                            v_sb,
                            vn_d[
                                b * SEQ : (b + 1) * SEQ, j * DH : (j + 1) * DH
                            ].rearrange("(c p) d -> p c d", p=128),
                        )
                        av_r = bav_sb.tile([128, SEQ], dmm, tag="avres")
                        avres[(b, j)] = av_r
                        for qt_i in range(4):
                            q0 = qt_i * 512
                            probs = bp.tile([128, 16, 512], dmm, tag="probs")
                            for kp in range(8):
                                sps = bs.tile([128, 2, 512], f32, tag="s")
                                for u in range(2):
                                    kt_i = 2 * kp + u
                                    nc.tensor.matmul(
                                        sps[:, u, :],
                                        kt_sb[:, kt_i * 128 : (kt_i + 1) * 128],
                                        qt_sb[:, q0 : q0 + 512],
                                        start=True,
                                        stop=True,
                                    )
                                nc.scalar.activation(
                                    probs[:, 2 * kp : 2 * kp + 2, :], sps, Exp
                                )
                            sum_ps = bsm.tile([1, 512], f32, tag="sum")
                            for kt_i in range(16):
                                nc.tensor.matmul(
                                    sum_ps,
                                    ones,
                                    probs[:, kt_i, :],
                                    start=(kt_i == 0),
                                    stop=(kt_i == 15),
                                )
                            sum_sb = bw.tile([1, 512], dmm, tag="sumsb")
                            nc.vector.tensor_copy(sum_sb, sum_ps)
                            rbc_ps = brb.tile([128, 512], f32, tag="rbc")
                            nc.tensor.matmul(rbc_ps, onesrow, sum_sb, start=True, stop=True)
                            rbc = bw.tile([128, 512], f32, tag="rbcsb")
                            nc.vector.reciprocal_approx_fast(rbc, rbc_ps)
                            avps = bav.tile([128, 512], f32, tag="av")
                            for cc in range(16):
                                nc.tensor.matmul(
                                    avps,
                                    v_sb[:, cc, :],
                                    probs[:, cc, :],
                                    start=(cc == 0),
                                    stop=(cc == 15),
                                )
                            nc.vector.tensor_mul(av_r[:, q0 : q0 + 512], avps, rbc)

                    # phase C for this batch: partial out = av^T.T @ wo_rows
                    for nl in range(SEQ // 128):
                        g0 = b * SEQ + nl * 128
                        ot = cot.tile([128, DIM], f32, tag="ot")
                        for do in range(4):
                            ops = bav.tile([128, 512], f32, tag="av")
                            for j in range(HL):
                                nc.tensor.matmul(
                                    ops,
                                    avres[(b, j)][:, nl * 128 : (nl + 1) * 128],
                                    wo_sb[:, j, do * 512 : (do + 1) * 512],
                                    start=(j == 0),
                                    stop=(j == 1),
                                )
                            nc.vector.tensor_copy(ot[:, do * 512 : (do + 1) * 512], ops)
                        nc.sync.dma_start(out_d[g0 : g0 + 128, :], ot)

    nc.compile()
    return nc


def _get_prog():
    mm_f32r = os.environ.get("KMM_DT", "f32r") == "f32r"
    key = ("prog", mm_f32r)
    if key not in _PROG:
        _PROG[key] = _build(mm_f32r)
    return _PROG[key], mm_f32r


def _shard(x, freqs_cis, wqkv, wo, mm_f32r):
    rnd = _round_f32r if mm_f32r else (lambda a: np.ascontiguousarray(a, np.float32))
    x = np.asarray(x, dtype=np.float32)
    freqs_cis = np.asarray(freqs_cis, dtype=np.float32)
    wqkv = np.asarray(wqkv, dtype=np.float32)
    wo = np.asarray(wo, dtype=np.float32)

    xt = rnd(x.reshape(NT, DIM).T)

    cos = freqs_cis[:, :, 0]  # [SEQ, 64]
    sin = freqs_cis[:, :, 1]
    cosb = np.concatenate([cos] * B, axis=0)  # [NT, 64], row n = b*SEQ + pos
    sinb = np.concatenate([sin] * B, axis=0)
    cos2 = rnd(np.concatenate([cosb, cosb], axis=1))  # [NT, 128] dup halves
    sin2 = rnd(np.concatenate([sinb, sinb], axis=1))

    perm = np.concatenate([np.arange(0, DH, 2), np.arange(1, DH, 2)])  # de-interleave
    consts = {
        "ident": np.eye(128, dtype=np.float32),
        "ones": np.ones((128, 1), np.float32),
        "onesrow": np.ones((1, 128), np.float32),
    }
    in_maps = []
    for c in range(NCORES):
        h0 = c * HL
        wq = [wqkv[:, h * DH : (h + 1) * DH][:, perm] * SCALE for h in (h0, h0 + 1)]
        wk = [wqkv[:, DIM + h * DH : DIM + (h + 1) * DH][:, perm] for h in (h0, h0 + 1)]
        wqk_c = rnd(np.concatenate(wq + wk, axis=1))  # [DIM, 512]
        wv_c = rnd(wqkv[:, 2 * DIM + h0 * DH : 2 * DIM + (h0 + HL) * DH])  # [DIM, 256]
        wo_c = rnd(wo[h0 * DH : (h0 + HL) * DH, :])  # [256, DIM]
        in_maps.append(
            {
                "xt": xt,
                "wqk": wqk_c,
                "wv": wv_c,
                "wo_r": wo_c,
                "cos2": cos2,
                "sin2": sin2,
                **consts,
            }
        )
    return in_maps


def _run(in_maps, trace=False, **kw):
    from concourse.bass_utils import run_bass_kernel_spmd

    prog, _ = _get_prog()
    return run_bass_kernel_spmd(prog, in_maps, list(range(NCORES)), trace=trace, **kw)


def kernel(x, freqs_cis, wqkv, wo):
    _, mm_f32r = _get_prog()
    in_maps = _shard(x, freqs_cis, wqkv, wo, mm_f32r)
    res = _run(in_maps, trace=False)
    acc = np.zeros((NT, DIM), dtype=np.float32)
    for c in range(NCORES):
        acc += res.results[c]["out_p"]
    return acc.reshape(B, SEQ, DIM)


# revision 5
# speedup vs baseline: 1.1734x; 1.0365x over previous
"""Trainium2 Bass kernel for nn_BidirectionalAttention (B=2, N=2048, D=2048, H=16).

Head-parallel tensor sharding across 8 NeuronCores (2 heads/core):
  phase A: qkv projection from x^T (rope applied on natural layout, then
           PE-transpose q,k into [head_dim, seq] layout), intermediates to DRAM
  phase B: per (batch, head): transposed attention scores s^T[k,q] = k^T.T @ q^T,
           exp on ScalarE, softmax denominator via ones-matmul partition sum
           (broadcast back to 128 partitions with a K=1 matmul, then fast
           reciprocal), unnormalized attn @ v accumulated transposed, scaled
  phase C: output projection partial = av^T.T @ wo_rows per core (interleaved
           with phase B per batch; av stays SBUF-resident)
Host: shard/transpose/pre-round inputs, sum the 8 partial outputs (the
"all-reduce after wo" done at gather time).

Matmuls run in float32r (tf32-like: 11-bit mantissa, full-rate PE) by default;
set KMM_DT=f32 for full-precision fp32 matmuls (4x slower PE).
"""

import os
import sys

sys.path.insert(0, "/opt/trn_rl_repo")

import numpy as np

B, SEQ, DIM, NHEAD, DH = 2, 2048, 2048, 16, 128
HL = NHEAD // 8  # heads per core = 2
NCORES = 8
NT = B * SEQ  # 4096 flattened rows
SCALE = 1.0 / np.sqrt(DH)

_PROG = {}


def _round_f32r(a):
    """Round fp32 array to fp32r (tf32-like): 8-bit exp, 11-bit stored mantissa,
    low 12 bits zero. Round-to-nearest-even."""
    b = np.ascontiguousarray(a, dtype=np.float32).view(np.uint32).astype(np.uint64)
    r = ((b + 0x7FF + ((b >> 12) & 1)) & np.uint64(0xFFFFF000)).astype(np.uint32)
    return r.view(np.float32)


def _build(mm_f32r: bool):
    import concourse.tile as tile
    from concourse import bacc, mybir

    f32 = mybir.dt.float32
    f32r = mybir.dt.float32r
    Exp = mybir.ActivationFunctionType.Exp
    dmm = f32r if mm_f32r else f32

    nc = bacc.Bacc("TRN2", target_bir_lowering=False, debug=False, num_devices=NCORES)

    xt_d = nc.dram_tensor("xt", [DIM, NT], dmm, kind="ExternalInput")
    wqk_d = nc.dram_tensor("wqk", [DIM, 4 * DH], dmm, kind="ExternalInput")
    wv_d = nc.dram_tensor("wv", [DIM, HL * DH], dmm, kind="ExternalInput")
    wo_d = nc.dram_tensor("wo_r", [HL * DH, DIM], dmm, kind="ExternalInput")
    cos_d = nc.dram_tensor("cos2", [128, 32 * 128], dmm, kind="ExternalInput")
    sin_d = nc.dram_tensor("sin2", [128, 32 * 128], dmm, kind="ExternalInput")
    ident_d = nc.dram_tensor("ident", [128, 128], dmm, kind="ExternalInput")
    ones_d = nc.dram_tensor("ones", [128, 1], dmm, kind="ExternalInput")
    onesrow_d = nc.dram_tensor("onesrow", [1, 128], dmm, kind="ExternalInput")
    out_d = nc.dram_tensor("out_p", [NT, DIM], f32, kind="ExternalOutput")

    qt_d = nc.dram_tensor("q_t", [HL, DH, NT], dmm)
    kt_d = nc.dram_tensor("k_t", [HL, DH, NT], dmm)
    vn_d = nc.dram_tensor("v_n", [NT, HL * DH], dmm)

    with tile.TileContext(nc) as tc:
        with (
            nc.allow_low_precision(reason="fp32r (tf32-like) matmul pipeline"),
            tc.tile_pool(name="const", bufs=1) as cp,
        ):
            ident = cp.tile([128, 128], dmm)
            ones = cp.tile([128, 1], dmm)
            onesrow = cp.tile([1, 128], dmm)
            wo_sb = cp.tile([128, HL, DIM], dmm)

            # ---------------- Phase A: qkv projection + rope + transpose ----
            with (
                tc.tile_pool(name="aconst", bufs=1) as ac,
                tc.tile_pool(name="axs", bufs=2) as axs,
                tc.tile_pool(name="awork", bufs=3) as aw,
                tc.tile_pool(name="apsum", bufs=2, space="PSUM") as aps,
                tc.tile_pool(name="atps", bufs=2, space="PSUM") as atp,
            ):
                wqk_sb = ac.tile([128, 16, 4 * DH], dmm)
                wqk_src = wqk_d.rearrange("(c p) m -> p c m", p=128)
                wv_sb = ac.tile([128, 16, HL * DH], dmm)
                wv_src = wv_d.rearrange("(c p) m -> p c m", p=128)
                for cc in range(16):
                    nc.sync.dma_start(wqk_sb[:, cc, :], wqk_src[:, cc, :])
                xt_all = xt_d.rearrange("(c p) n -> p c n", p=128)
                xs0 = axs.tile([128, 16, 512], dmm, tag="xs")
                for cg in range(4):
                    nc.sync.dma_start(
                        xs0[:, 4 * cg : 4 * cg + 4, :], xt_all[:, 4 * cg : 4 * cg + 4, 0:512]
                    )
                for cc in range(16):
                    nc.sync.dma_start(wv_sb[:, cc, :], wv_src[:, cc, :])
                cos_sb = ac.tile([128, 32, 128], dmm)
                nc.sync.dma_start(cos_sb.rearrange("p i f -> p (i f)"), cos_d[:, :])
                sin_sb = ac.tile([128, 32, 128], dmm)
                nc.sync.dma_start(sin_sb.rearrange("p i f -> p (i f)"), sin_d[:, :])
                nc.sync.dma_start(ident, ident_d[:, :])
                nc.sync.dma_start(ones, ones_d[:, :])
                nc.sync.dma_start(onesrow, onesrow_d[:, :])
                nc.sync.dma_start(wo_sb, wo_d.rearrange("(j p) o -> p j o", p=128))

                for sl in range(NT // 512):
                    n0 = sl * 512
                    if sl == 0:
                        xs = xs0
                    else:
                        xs = axs.tile([128, 16, 512], dmm, tag="xs")
                        xt_src = xt_all[:, :, n0 : n0 + 512]
                        for cg in range(4):
                            nc.sync.dma_start(
                                xs[:, 4 * cg : 4 * cg + 4, :], xt_src[:, 4 * cg : 4 * cg + 4, :]
                            )
                    tqk = [aw.tile([128, 512], dmm, tag=f"tqk{t}", name=f"tqk{t}") for t in range(4)]
                    for st in range(4):
                        i = sl * 4 + st
                        g0 = n0 + st * 128
                        xsl = xs[:, :, st * 128 : (st + 1) * 128]
                        qkps = aps.tile([128, 4 * DH], f32, tag="qk")
                        for cc in range(16):
                            nc.tensor.matmul(
                                qkps,
                                xsl[:, cc, :],
                                wqk_sb[:, cc, :],
                                start=(cc == 0),
                                stop=(cc == 15),
                            )
                        vps = aps.tile([128, HL * DH], f32, tag="v")
                        for cc in range(16):
                            nc.tensor.matmul(
                                vps,
                                xsl[:, cc, :],
                                wv_sb[:, cc, :],
                                start=(cc == 0),
                                stop=(cc == 15),
                            )
                        vsb = aw.tile([128, HL * DH], dmm, tag="vsb")
                        nc.scalar.copy(vsb, vps)
                        nc.sync.dma_start(vn_d[g0 : g0 + 128, :], vsb)

                        # rope: cols [q0 q1 k0 k1], each 128 = [64 even | 64 odd]
                        rt = aw.tile([128, 4 * DH], dmm, tag="rt")
                        ca = cos_sb[:, i, :].rearrange("p (t f) -> p t f", t=2)
                        sa = sin_sb[:, i, :].rearrange("p (t f) -> p t f", t=2)
                        for g in range(2):
                            blk = qkps[:, g * 256 : (g + 1) * 256].rearrange(
                                "p (t h f) -> p t h f", t=2, h=2
                            )
                            rbl = rt[:, g * 256 : (g + 1) * 256].rearrange(
                                "p (t h f) -> p t h f", t=2, h=2
                            )
                            ev, od = blk[:, :, 0, :], blk[:, :, 1, :]
                            tA = aw.tile([128, 2, 64], f32, tag="tA")
                            tB = aw.tile([128, 2, 64], f32, tag="tB")
                            nc.vector.tensor_mul(tA, od, sa)
                            nc.vector.tensor_mul(tB, ev, ca)
                            nc.vector.tensor_sub(rbl[:, :, 0, :], tB, tA)
                            tC = aw.tile([128, 2, 64], f32, tag="tC")
                            tD = aw.tile([128, 2, 64], f32, tag="tD")
                            nc.vector.tensor_mul(tC, ev, sa)
                            nc.vector.tensor_mul(tD, od, ca)
                            nc.vector.tensor_add(rbl[:, :, 1, :], tD, tC)

                        for t in range(4):
                            tp = atp.tile([128, 128], dmm, tag="tp")
                            nc.tensor.transpose(tp, rt[:, t * 128 : (t + 1) * 128], ident)
                            nc.scalar.copy(tqk[t][:, st * 128 : (st + 1) * 128], tp)
                    for t in range(4):
                        dst = (qt_d if t < 2 else kt_d)[t % 2, :, n0 : n0 + 512]
                        nc.sync.dma_start(dst, tqk[t])

            # ---------- Phase B+C: attention + output projection ------------
            with (
                tc.tile_pool(name="bin", bufs=2) as bi,
                tc.tile_pool(name="bprobs", bufs=2) as bp,
                tc.tile_pool(name="bwork", bufs=3) as bw,
                tc.tile_pool(name="bavres", bufs=3) as bav_sb,
                tc.tile_pool(name="cot", bufs=2) as cot,
                tc.tile_pool(name="bs", bufs=2, space="PSUM") as bs,
                tc.tile_pool(name="bsum", bufs=1, space="PSUM") as bsm,
                tc.tile_pool(name="brb", bufs=1, space="PSUM") as brb,
                tc.tile_pool(name="bav", bufs=2, space="PSUM") as bav,
            ):
                avres = {}
                for b in range(B):
                    for j in range(HL):
                        kt_sb = bi.tile([128, SEQ], dmm, tag="kt")
                        nc.sync.dma_start(kt_sb, kt_d[j, :, b * SEQ : (b + 1) * SEQ])
                        qt_sb = bi.tile([128, SEQ], dmm, tag="qt")
                        nc.sync.dma_start(qt_sb, qt_d[j, :, b * SEQ : (b + 1) * SEQ])
                        v_sb = bi.tile([128, 16, DH], dmm, tag="v")
                        nc.sync.dma_start(
                            v_sb,
                            vn_d[
                                b * SEQ : (b + 1) * SEQ, j * DH : (j + 1) * DH
                            ].rearrange("(c p) d -> p c d", p=128),
                        )
                        av_r = bav_sb.tile([128, SEQ], dmm, tag="avres")
                        avres[(b, j)] = av_r
                        for qt_i in range(4):
                            q0 = qt_i * 512
                            probs = bp.tile([128, 16, 512], dmm, tag="probs")
                            for kp in range(8):
                                sps = bs.tile([128, 2, 512], f32, tag="s")
                                for u in range(2):
                                    kt_i = 2 * kp + u
                                    nc.tensor.matmul(
                                        sps[:, u, :],
                                        kt_sb[:, kt_i * 128 : (kt_i + 1) * 128],
                                        qt_sb[:, q0 : q0 + 512],
                                        start=True,
                                        stop=True,
                                    )
                                nc.scalar.activation(
                                    probs[:, 2 * kp : 2 * kp + 2, :], sps, Exp
                                )
                            sum_ps = bsm.tile([1, 512], f32, tag="sum")
                            for kt_i in range(16):
                                nc.tensor.matmul(
                                    sum_ps,
                                    ones,
                                    probs[:, kt_i, :],
                                    start=(kt_i == 0),
                                    stop=(kt_i == 15),
                                )
                            sum_sb = bw.tile([1, 512], dmm, tag="sumsb")
                            nc.vector.tensor_copy(sum_sb, sum_ps)
                            rbc_ps = brb.tile([128, 512], f32, tag="rbc")
                            nc.tensor.matmul(rbc_ps, onesrow, sum_sb, start=True, stop=True)
                            rbc = bw.tile([128, 512], f32, tag="rbcsb")
                            nc.vector.reciprocal_approx_fast(rbc, rbc_ps)
                            avps = bav.tile([128, 512], f32, tag="av")
                            for cc in range(16):
                                nc.tensor.matmul(
                                    avps,
                                    v_sb[:, cc, :],
                                    probs[:, cc, :],
                                    start=(cc == 0),
                                    stop=(cc == 15),
                                )
                            nc.vector.tensor_mul(av_r[:, q0 : q0 + 512], avps, rbc)

                    # phase C for this batch: partial out = av^T.T @ wo_rows
                    for nl in range(SEQ // 128):
                        g0 = b * SEQ + nl * 128
                        ot = cot.tile([128, DIM], f32, tag="ot")
                        for do in range(4):
                            ops = bav.tile([128, 512], f32, tag="av")
                            for j in range(HL):
                                nc.tensor.matmul(
                                    ops,
                                    avres[(b, j)][:, nl * 128 : (nl + 1) * 128],
                                    wo_sb[:, j, do * 512 : (do + 1) * 512],
                                    start=(j == 0),
                                    stop=(j == 1),
                                )
                            nc.vector.tensor_copy(ot[:, do * 512 : (do + 1) * 512], ops)
                        nc.sync.dma_start(out_d[g0 : g0 + 128, :], ot)

    nc.compile()
    return nc


def _get_prog():
    mm_f32r = os.environ.get("KMM_DT", "f32r") == "f32r"
    key = ("prog", mm_f32r)
    if key not in _PROG:
        _PROG[key] = _build(mm_f32r)
    return _PROG[key], mm_f32r


def _shard(x, freqs_cis, wqkv, wo, mm_f32r):
    rnd = _round_f32r if mm_f32r else (lambda a: np.ascontiguousarray(a, np.float32))
    x = np.asarray(x, dtype=np.float32)
    freqs_cis = np.asarray(freqs_cis, dtype=np.float32)
    wqkv = np.asarray(wqkv, dtype=np.float32)
    wo = np.asarray(wo, dtype=np.float32)

    xt = rnd(x.reshape(NT, DIM).T)

    cos = freqs_cis[:, :, 0]  # [SEQ, 64]
    sin = freqs_cis[:, :, 1]
    cosb = np.concatenate([cos] * B, axis=0)  # [NT, 64], row n = b*SEQ + pos
    sinb = np.concatenate([sin] * B, axis=0)
    cos2n = np.concatenate([cosb, cosb], axis=1)  # [NT, 128] dup halves
    sin2n = np.concatenate([sinb, sinb], axis=1)
    # partition-major for contiguous DMA: [128 p, 32 i, 128 f] flattened
    cos2 = rnd(cos2n.reshape(32, 128, 128).transpose(1, 0, 2).reshape(128, 32 * 128))
    sin2 = rnd(sin2n.reshape(32, 128, 128).transpose(1, 0, 2).reshape(128, 32 * 128))

    perm = np.concatenate([np.arange(0, DH, 2), np.arange(1, DH, 2)])  # de-interleave
    consts = {
        "ident": np.eye(128, dtype=np.float32),
        "ones": np.ones((128, 1), np.float32),
        "onesrow": np.ones((1, 128), np.float32),
    }
    in_maps = []
    for c in range(NCORES):
        h0 = c * HL
        wq = [wqkv[:, h * DH : (h + 1) * DH][:, perm] * SCALE for h in (h0, h0 + 1)]
        wk = [wqkv[:, DIM + h * DH : DIM + (h + 1) * DH][:, perm] for h in (h0, h0 + 1)]
        wqk_c = rnd(np.concatenate(wq + wk, axis=1))  # [DIM, 512]
        wv_c = rnd(wqkv[:, 2 * DIM + h0 * DH : 2 * DIM + (h0 + HL) * DH])  # [DIM, 256]
        wo_c = rnd(wo[h0 * DH : (h0 + HL) * DH, :])  # [256, DIM]
        in_maps.append(
            {
                "xt": xt,
                "wqk": wqk_c,
                "wv": wv_c,
                "wo_r": wo_c,
                "cos2": cos2,
                "sin2": sin2,
                **consts,
            }
        )
    return in_maps


def _run(in_maps, trace=False, **kw):
    from concourse.bass_utils import run_bass_kernel_spmd

    prog, _ = _get_prog()
    return run_bass_kernel_spmd(prog, in_maps, list(range(NCORES)), trace=trace, **kw)


def kernel(x, freqs_cis, wqkv, wo):
    _, mm_f32r = _get_prog()
    in_maps = _shard(x, freqs_cis, wqkv, wo, mm_f32r)
    res = _run(in_maps, trace=False)
    acc = np.zeros((NT, DIM), dtype=np.float32)
    for c in range(NCORES):
        acc += res.results[c]["out_p"]
    return acc.reshape(B, SEQ, DIM)
